# revision 13
# baseline (speedup 1.0000x reference)
"""FM layer (first + second order factorization machine) on 8 TRN2 NeuronCores.

Strategy: batch-parallel. The full embedding table (w concatenated with V^T,
[2_600_013, 17] fp32) is replicated to every core's HBM; each core handles
512 of the 4096 batch rows. Per core the kernel gathers 512*26 rows of 68B
via indirect DMA, reduces over the 26 fields, and combines with the dense
contribution computed by one small matmul per 128-row tile. No collectives.

Math (per batch row b, fields f=1..26, latent dim k=1..16):
  idx[b,f] = sparse[b,f] + 13 + f*100000
  first  = w0 + d@wd + sum_f w[idx]
  e      = d@Vd + sum_f Vt[idx]          (16-vector)
  sq     = d^2@Vd^2 + sum_f Vt[idx]^2    (16-vector)
  out    = first + 0.5*(sum_k e^2 - sum_k sq)
The dense part is folded into one matmul: lhsT = [d^T; (d^2)^T; ones] (27 x 128),
rhs (27 x 18) = [[wd | Vd | 0], [0 | 0 | rowsum(Vd^2)], [w0 | 0 | 0]], so
mm[:, 0] = w0 + d@wd, mm[:, 1:17] = d@Vd, mm[:, 17] = sum_k (d^2@Vd^2)[k].
"""

import os
import sys

sys.path.insert(0, "/opt/trn_rl_repo")

import numpy as np

import concourse.bass as bass
import concourse.mybir as mybir
import concourse.tile as tile

N_DENSE = 13
N_FIELDS = 26
PER_FIELD = 100000
FEATURE_NUM = N_FIELDS * PER_FIELD + N_DENSE  # 2_600_013
K = 16
BATCH = 4096
N_CORES = 8
BL = BATCH // N_CORES  # 512 batch rows per core
P = 128
T = BL // P  # 4 tiles of 128 rows per core
ROW = 1 + K  # 17 floats per table row (w | V^T row)
KM = 2 * N_DENSE + 1  # 27 matmul contraction rows
NO = ROW + 1  # 18 matmul output cols

F32 = mybir.dt.float32
I32 = mybir.dt.int32

# How many 128-row batch tiles share one indirect-DMA gather instruction.
# (1 => 4 gathers of 3328 rows; 2 => 2 gathers of 6656 rows; 4 => 1 of 13312)
GATHER_TILES = 2


def split_multiwaits(nc: bass.Bass, max_waits: int = 1) -> int:
    """This container's walrus encodes at most one sync-wait per instruction
    (setupSyncWait raises 'Too many sync wait commands' otherwise). Hoist
    extra waits into standalone EventSemaphore ops on the same engine.
    Each hoisted op incs a dedicated dummy sem nothing waits on (CoreSim
    requires EventSemaphore instructions to carry an update)."""
    import bass_rust

    # Tile assigns its sems (ids ~151-168) outside bass's free pool, so pick
    # the first bass-free id above everything Tile used.
    used = set()
    for func in nc.m.functions:
        for bb in func.blocks:
            for ins in bb.instructions:
                si = getattr(ins, "sync_info", None)
                if si:
                    for x in list(si.on_wait or []) + list(si.on_update or []):
                        used.add(x.id)
    dummy = None
    for num in range(max(used, default=0) + 1, 256):
        try:
            dummy = nc.alloc_semaphore("splitw_dummy", num=num)
            break
        except AssertionError:
            continue
    assert dummy is not None, "no free semaphore for splitw_dummy"
    n = 0
    for func in nc.m.functions:
        for bb in func.blocks:
            out = []
            for ins in bb.instructions:
                si = getattr(ins, "sync_info", None)
                if (
                    si is not None
                    and si.on_wait is not None
                    and len(si.on_wait) > max_waits
                ):
                    for w in list(si.on_wait[:-max_waits]):
                        n += 1
                        ev = mybir.InstEventSemaphore(
                            name=f"splitw_{n}", engine=ins.engine
                        )
                        ev.sync_info = mybir.SyncInfo(on_wait=[w], on_update=[])
                        bass_rust.then_inc(ev, dummy, 1, True)
                        out.append(ev)
                    ins.sync_info = mybir.SyncInfo(
                        on_wait=list(si.on_wait[-max_waits:]),
                        on_update=list(si.on_update or []),
                    )
                out.append(ins)
            bb.instructions = out
    return n


def build_nc() -> bass.Bass:
    nc = bass.Bass()

    table = nc.dram_tensor("table", [FEATURE_NUM, ROW], F32, kind="ExternalInput")
    idx = nc.dram_tensor("idx", [P, T * N_FIELDS], I32, kind="ExternalInput")
    # dense matmul operands packed in one tensor: cols 0..BL-1 = lhsT,
    # cols BL..BL+NO-1 = rhs
    dmat = nc.dram_tensor("dmat", [KM, BL + NO], F32, kind="ExternalInput")
    out = nc.dram_tensor("out", [P, T], F32, kind="ExternalOutput")

    with tile.TileContext(nc) as tc:
        with (
            tc.tile_pool(name="const", bufs=1) as cp,
            tc.tile_pool(name="sbuf", bufs=2) as sp,
            tc.tile_pool(name="psum", bufs=2, space="PSUM") as pp,
        ):
            idx_t = cp.tile([P, T * N_FIELDS], I32)
            nc.sync.dma_start(idx_t[:], idx[:])
            dmat_t = cp.tile([KM, BL + NO], F32)
            nc.sync.dma_start(dmat_t[:], dmat[:])
            out_t = cp.tile([P, T], F32)

            # all dense matmuls upfront: mm_all[:, t*NO:(t+1)*NO] for tile t
            mm_all = pp.tile([P, T * NO], F32)
            for t in range(T):
                nc.tensor.matmul(
                    mm_all[:, t * NO : (t + 1) * NO],
                    dmat_t[:, t * P : (t + 1) * P],
                    dmat_t[:, BL : BL + NO],
                    start=True,
                    stop=True,
                )

            # Per-field gathers: HW indirect DMA supports exactly one index
            # per partition per instruction (the ucode reads idx[p, 0] and
            # fetches out.free_size contiguous elements), so one instruction
            # per (tile, field).
            for t in range(T):
                gt_tile = sp.tile([P, N_FIELDS * ROW], F32, tag="g")
                for f in range(N_FIELDS):
                    nc.gpsimd.indirect_dma_start(
                        out=gt_tile[:, f * ROW : (f + 1) * ROW],
                        out_offset=None,
                        in_=table[:],
                        in_offset=bass.IndirectOffsetOnAxis(
                            ap=idx_t[:, t * N_FIELDS + f : t * N_FIELDS + f + 1],
                            axis=0,
                        ),
                    )
                if True:
                    gt = gt_tile[:]

                    # sum over fields: [P, 17] (col 0 = sum w, 1:17 = sum V rows)
                    sf = sp.tile([P, ROW], F32, tag="sf")
                    nc.vector.tensor_reduce(
                        out=sf[:],
                        in_=gt.rearrange("p (f c) -> p c f", f=N_FIELDS),
                        axis=mybir.AxisListType.X,
                        op=mybir.AluOpType.add,
                    )

                    # sum of squares of gathered V rows: [P, 1]
                    s2 = sp.tile([P, 1], F32, tag="s2")
                    sqs = sp.tile([P, N_FIELDS * K], F32, tag="sqs")
                    nc.scalar.activation(
                        out=sqs[:].rearrange("p (f c) -> p f c", f=N_FIELDS),
                        in_=gt.rearrange("p (f c) -> p f c", f=N_FIELDS)[:, :, 1:ROW],
                        func=mybir.ActivationFunctionType.Square,
                        accum_out=s2[:],
                    )

                    # t = sparse sums + dense part: col0 = first order, 1:17 = e
                    mm = mm_all[:, t * NO : (t + 1) * NO]
                    ts = sp.tile([P, ROW], F32, tag="ts")
                    nc.vector.tensor_tensor(
                        out=ts[:], in0=sf[:], in1=mm[:, 0:ROW],
                        op=mybir.AluOpType.add,
                    )

                    # sum_k e^2
                    se2 = sp.tile([P, 1], F32, tag="se2")
                    sq2 = sp.tile([P, K], F32, tag="sq2")
                    nc.scalar.activation(
                        out=sq2[:],
                        in_=ts[:, 1:ROW],
                        func=mybir.ActivationFunctionType.Square,
                        accum_out=se2[:],
                    )

                    # out = ts[:,0] + 0.5*(se2 - s2 - mm[:,17])
                    d1 = sp.tile([P, 1], F32, tag="d1")
                    nc.vector.tensor_tensor(
                        out=d1[:], in0=se2[:], in1=s2[:],
                        op=mybir.AluOpType.subtract,
                    )
                    d2 = sp.tile([P, 1], F32, tag="d2")
                    nc.vector.tensor_tensor(
                        out=d2[:], in0=d1[:], in1=mm[:, ROW : ROW + 1],
                        op=mybir.AluOpType.subtract,
                    )
                    nc.vector.tensor_scalar(
                        out=out_t[:, t : t + 1],
                        in0=d2[:],
                        scalar1=0.5,
                        scalar2=ts[:, 0:1],
                        op0=mybir.AluOpType.mult,
                        op1=mybir.AluOpType.add,
                    )

            nc.sync.dma_start(out[:], out_t[:])

    split_multiwaits(nc)
    return nc


def prepare_inputs(dense_inputs, sparse_inputs, w0, w, V):
    """Host-side preprocessing -> per-core input maps."""
    dense = np.asarray(dense_inputs, dtype=np.float32)
    sparse = np.asarray(sparse_inputs, dtype=np.int32)
    w0 = np.asarray(w0, dtype=np.float32).reshape(-1)
    w = np.asarray(w, dtype=np.float32).reshape(FEATURE_NUM, 1)
    V = np.asarray(V, dtype=np.float32)

    table = np.concatenate([w, V.T], axis=1)  # [FEATURE_NUM, 17]
    table = np.ascontiguousarray(table, dtype=np.float32)

    offsets = (N_DENSE + np.arange(N_FIELDS, dtype=np.int32) * PER_FIELD).astype(
        np.int32
    )
    gidx = sparse + offsets[None, :]  # [B, 26] global row ids

    wd = w[:N_DENSE, 0]  # [13]
    Vd = V[:, :N_DENSE].T.astype(np.float32)  # [13, 16]
    u = (Vd * Vd).sum(axis=1)  # [13]

    rhs = np.zeros((KM, NO), dtype=np.float32)
    rhs[:N_DENSE, 0] = wd
    rhs[:N_DENSE, 1:ROW] = Vd
    rhs[N_DENSE : 2 * N_DENSE, ROW] = u
    rhs[2 * N_DENSE, 0] = w0[0]

    in_maps = []
    for c in range(N_CORES):
        dslice = dense[c * BL : (c + 1) * BL]  # [512, 13]
        dmat = np.empty((KM, BL + NO), dtype=np.float32)
        dmat[:N_DENSE, :BL] = dslice.T
        dmat[N_DENSE : 2 * N_DENSE, :BL] = (dslice * dslice).T
        dmat[2 * N_DENSE, :BL] = 1.0
        dmat[:, BL:] = rhs

        gslice = gidx[c * BL : (c + 1) * BL]  # [512, 26]
        # idx_arr[p, t*26+f] = gidx[c*512 + t*128 + p, f]
        idx_arr = np.ascontiguousarray(
            gslice.reshape(T, P, N_FIELDS).transpose(1, 0, 2).reshape(P, T * N_FIELDS)
        ).astype(np.int32)

        in_maps.append({"table": table, "idx": idx_arr, "dmat": dmat})
    return in_maps


def assemble_output(results):
    """Per-core [128, 4] outputs -> [4096, 1]."""
    out = np.empty((BATCH, 1), dtype=np.float32)
    for c in range(N_CORES):
        o = results[c]["out"]  # [128, T]; out[p, t] = row c*512 + t*128 + p
        out[c * BL : (c + 1) * BL, 0] = o.T.reshape(BL)
    return out


_NC_CACHE = None


def kernel(**inputs) -> np.ndarray:
    global _NC_CACHE
    from concourse.bass_utils import run_bass_kernel_spmd

    if _NC_CACHE is None:
        _NC_CACHE = build_nc()
    nc = _NC_CACHE
    in_maps = prepare_inputs(**inputs)
    res = run_bass_kernel_spmd(nc, in_maps, list(range(N_CORES)))
    return assemble_output(res.results)
